# revision 11
# baseline (speedup 1.0000x reference)
"""Causal self-attention (B=2, N=2048, D=1024, H=16, hd=64) on 8 trn2 NeuronCores.

Sharding: core c handles batch b = c//4 and 4 heads hs = [4*(c%4) .. 4*(c%4)+3]
(tensor-parallel over heads x data-parallel over batch). Each core computes its
heads' attention and a row-parallel partial of the output projection
(partial[n, :] = sum_{local heads} sa_h[n, :] @ Wo[h*64:(h+1)*64, :]); the host
sums the 4 partials per batch and adds the output bias.

Schedule (v2): kt-interleaved input DMA so the first projection matmuls start
~2us in (kt-partial accumulation of v/qk for query-block 0 as chunks land);
attention emitted as (t, pair) units with a 2-unit acc lag so the Exp latency
hides behind the next unit's score matmuls; later-block projections and the
output projection are woven between attention units so the PE never drains
while the Act engine (Exp) catches up. v-bias folded into the PSUM->SBUF drain
(DVE add with a partition-broadcast bias tile) instead of PE matmuls; the d=3
diagonal score/accumulate matmuls run 256 wide (fp32r <256-wide pays 4x) with
a [zeros|tri] mask zeroing the extra columns.
"""

import numpy as np
import ml_dtypes
from contextlib import ExitStack

import concourse.bass as bass
import concourse.tile as tile
from concourse import bacc, mybir
from concourse import bass_utils

F32 = mybir.dt.float32
F32R = mybir.dt.float32r
BF16 = mybir.dt.bfloat16
EXP = mybir.ActivationFunctionType.Exp

B, N, D, H, HD = 2, 2048, 1024, 16, 64
N_CORES = 8
LH = 4            # local heads per core
KT = D // 128     # 8 contraction k-tiles
NT = N // 128     # 16 n-tiles
NB = N // 512     # 4 n-blocks / q-blocks
QB = 512

_CACHE: dict = {}

CFG = {
    "at_bufs": 4,
    "oe_bufs": 3,
    "sc_bufs": 2,
    "acc_lag": 2,
    "act_primer": True,
    "n_warmup": 10,
}


def _emit(nc, tc, ctx, io, repeat=1, dbg=None):
    xT, wqk, wv, bqk, bv, wo, tri2, out = io

    persist = ctx.enter_context(tc.tile_pool(name="persist", bufs=1))
    sbp = ctx.enter_context(tc.tile_pool(name="work", bufs=1))
    psum = ctx.enter_context(tc.tile_pool(name="psum", bufs=1, space="PSUM"))

    # ---- persistent SBUF tensors ----
    xT_sb = persist.tile([128, KT, N], BF16)
    wqk_sb = persist.tile([128, KT, 512], BF16)
    wv_sb = persist.tile([128, KT, 256], BF16)
    wo_sb = persist.tile([128, 2, 1024], F32R)
    bqk_sb = persist.tile([128, 4], F32)
    bv_sb = persist.tile([1, 256], F32)
    bvbc_sb = persist.tile([128, 256], F32)
    ones_sb = persist.tile([1, 128], F32)
    tri2_sb = persist.tile([128, 256], F32R)
    qkT_sb = persist.tile([128, 4, N], F32R)
    v65_sb = persist.tile([128, NT, LH * 65], F32R)
    saT_sb = persist.tile([128, 2, N], F32R)

    # ---- input DMAs (SP issue order == DMA service order) ----
    for kt in range(KT):
        nc.sync.dma_start(wv_sb[:, kt, :], wv[kt * 128:(kt + 1) * 128, :])
        nc.sync.dma_start(xT_sb[:, kt, 0:QB], xT[kt * 128:(kt + 1) * 128, 0:QB])
        nc.sync.dma_start(wqk_sb[:, kt, :], wqk[kt * 128:(kt + 1) * 128, :])
        if kt == 1:
            nc.sync.dma_start(bqk_sb[:], bqk.rearrange("t p -> p t"))
            nc.sync.dma_start(bv_sb[:], bv[:])
            nc.sync.dma_start(tri2_sb[:], tri2[:])
    for nb in range(1, NB):
        nc.sync.dma_start(
            xT_sb[:, :, nb * QB:(nb + 1) * QB],
            xT[:, nb * QB:(nb + 1) * QB].rearrange("(kt p) c -> p kt c", p=128),
        )
    for kt2 in range(2):
        nc.sync.dma_start(wo_sb[:, kt2, :], wo[kt2 * 128:(kt2 + 1) * 128, :])

    nc.vector.memset(ones_sb[:], 1.0)
    # the +1 denominator row of v65: free index h*65+64 per (nt, h)
    ones_col = v65_sb[:, :, :].rearrange("p n (h c) -> p n h c", c=65)[:, :, :, 64:65]
    nc.vector.memset(ones_col.bitcast(F32), 1.0)
    # broadcast v-bias across partitions once (Pool is idle)
    nc.gpsimd.partition_broadcast(bvbc_sb[:], bv_sb[:])
    if CFG["act_primer"]:
        primer = sbp.tile([1, 1], F32, name="t_primer", tag="primer", bufs=1)
        nc.scalar.activation(primer[:], ones_sb[0:1, 0:1], EXP)
    if CFG["n_warmup"]:
        # ramp the PE p-state while the first input chunks stream in: dummy
        # rank-1 matmuls on a zeroed row into a never-read psum bank
        zz = persist.tile([1, 512], F32R)
        nc.vector.memset(zz[:].bitcast(F32), 0.0)
        wps = psum.tile([128, 512], F32, name="ps_op", tag="op", bufs=2)
        for i in range(CFG["n_warmup"]):
            nc.tensor.matmul(
                wps[:], ones_sb[0:1, 0:128].bitcast(F32R), zz[:],
                start=(i == 0), stop=(i == CFG["n_warmup"] - 1),
                skip_group_check=True,
            )

    # ---- phase pieces ----
    def v_mm(ps, nt, kt):
        # one 256-wide v matmul; a psum bank holds exactly one group
        # (start=True re-zeroes the whole 2KB zero region, so banks can't be
        # shared between accumulation groups)
        nc.tensor.matmul(
            ps[:, 0:256],
            xT_sb[:, kt, nt * 128:(nt + 1) * 128],
            wv_sb[:, kt, :],
            start=(kt == 0), stop=(kt == KT - 1),
            skip_group_check=True,
        )

    def v_drain(ps, nt):
        # psum -> v65 with the v-bias folded in (bias varies along free dim)
        dst = v65_sb[:, nt, :].rearrange("p (h c) -> p h c", c=65)[:, :, 0:64]
        src = ps[:, 0:256].rearrange("p (h c) -> p h c", c=64)
        bia = bvbc_sb[:, :].rearrange("p (h c) -> p h c", c=64)
        nc.vector.tensor_add(dst, src, bia)

    def qk_ct_mm(ps, nb, ct, kts):
        for kt in kts:
            nc.tensor.matmul(
                ps[:], wqk_sb[:, kt, ct * 128:(ct + 1) * 128],
                xT_sb[:, kt, nb * QB:(nb + 1) * QB],
                start=(kt == 0), stop=(kt == KT - 1),
                skip_group_check=True,
            )

    def qk_ct_drain(ps, nb, ct, eng="vector"):
        if eng == "scalar":
            nc.scalar.activation(
                qkT_sb[:, ct, nb * QB:(nb + 1) * QB], ps[:],
                mybir.ActivationFunctionType.Identity,
                bias=bqk_sb[:, ct:ct + 1])
        else:
            nc.vector.tensor_scalar_add(
                qkT_sb[:, ct, nb * QB:(nb + 1) * QB], ps[:], bqk_sb[:, ct:ct + 1])

    # steady-state filler pieces (each ~0.4-1 us of PE work)
    def fill_v(nt):
        ps = psum.tile([128, 512], F32, name="ps_op", tag="op", bufs=2)
        def h1():
            for kt in range(4):
                v_mm(ps, nt, kt)
        def h2():
            for kt in range(4, KT):
                v_mm(ps, nt, kt)
            v_drain(ps, nt)
        return [h1, h2]

    def fill_qk_ct(nb, ct):
        ps = psum.tile([128, 512], F32, name="ps_op", tag="op", bufs=2)
        def h1():
            qk_ct_mm(ps, nb, ct, range(4))
        def h2():
            qk_ct_mm(ps, nb, ct, range(4, KT))
            qk_ct_drain(ps, nb, ct)
        return [h1, h2]

    def fill_op(J, nqs, copy_eng="vector", dma_per_dh=False):
        # one output-projection row-block piece: both dh halves -> one DMA
        r0 = J * QB + nqs * 128
        oe = sbp.tile([128, 1024], BF16, name="t_oe", tag="oe", bufs=CFG["oe_bufs"])
        def piece(dh):
            def f():
                op = psum.tile([128, 512], F32, name="ps_op", tag="op", bufs=2)
                for kt2 in range(2):
                    nc.tensor.matmul(
                        op[:], saT_sb[:, kt2, r0:r0 + 128],
                        wo_sb[:, kt2, dh * 512:(dh + 1) * 512],
                        start=(kt2 == 0), stop=(kt2 == 1),
                        skip_group_check=True,
                    )
                if copy_eng == "scalar":
                    nc.scalar.copy(oe[:, dh * 512:(dh + 1) * 512], op[:])
                else:
                    nc.vector.tensor_copy(oe[:, dh * 512:(dh + 1) * 512], op[:])
                if dma_per_dh:
                    nc.sync.dma_start(out[r0:r0 + 128, dh * 512:(dh + 1) * 512],
                                      oe[:, dh * 512:(dh + 1) * 512])
                elif dh == 1:
                    nc.sync.dma_start(out[r0:r0 + 128, :], oe[:])
            return f
        return [piece(0), piece(1)]

    # ---- attention ----
    def emit_acc(J, accs, unit, n_t):
        t, p, at, c0e = unit
        for s in range(2):
            nc.tensor.matmul(
                accs[p][s][0:65, c0e:512],
                v65_sb[:, t, (2 * p + s) * 65:(2 * p + s) * 65 + 65],
                at[:, s * 512 + c0e:(s + 1) * 512],
                start=(t == 0), stop=(t == n_t - 1),
                skip_group_check=True,
            )

    def normalize(J, accs, p):
        for s in range(2):
            rc = sbp.tile([1, 512], F32, name="t_rc", tag="rc", bufs=2)
            nc.vector.reciprocal(rc[:], accs[p][s][64:65, :])
            bc = sbp.tile([64, 512], F32, name="t_bc", tag="bc", bufs=2)
            nc.gpsimd.partition_broadcast(bc[:], rc[:])
            nc.vector.tensor_mul(
                saT_sb[s * 64:(s + 1) * 64, p, J * QB:(J + 1) * QB],
                accs[p][s][0:64, :], bc[:],
            )

    def attention(J, fillers):
        n_t = 4 * J + 4
        n_units = n_t * 2
        n_fill = len(fillers)
        fill_i = 0
        unit_i = 0
        for p in range(2):
            accs = {p: [psum.tile([128, 512], F32, name="ps_acc", tag="acc",
                                  bufs=2) for _ in range(2)]}
            pend = []
            for t in range(n_t):
                d = t - 4 * J
                c0 = max(d, 0) * 128
                c0e = min(c0, 256)
                sc = psum.tile([128, 1024], F32, name="ps_sc", tag="sc",
                               bufs=CFG["sc_bufs"])
                for s in range(2):
                    nc.tensor.matmul(
                        sc[:, s * 512 + c0e:(s + 1) * 512],
                        qkT_sb[s * 64:(s + 1) * 64, 2 * p, t * 128:(t + 1) * 128],
                        qkT_sb[s * 64:(s + 1) * 64, 2 * p + 1, J * QB + c0e:(J + 1) * QB],
                        start=True, stop=True,
                    )
                at = sbp.tile([128, 1024], F32R, name="t_at", tag="at",
                              bufs=CFG["at_bufs"])
                nc.scalar.activation(at[:, c0e:1024], sc[:, c0e:1024],
                                     EXP, scale=0.125)
                if d == 3:
                    atv = at[:, 256:1024].rearrange(
                        "p (s c) -> p s c", c=256)[:, ::2, :]
                    nc.vector.tensor_mul(
                        atv, atv, tri2_sb[:, None, :].broadcast_to([128, 2, 256]))
                elif d >= 0:
                    atv = at[:, c0:c0 + 640].rearrange(
                        "p (s c) -> p s c", c=128)[:, ::4, :]
                    nc.vector.tensor_mul(
                        atv, atv,
                        tri2_sb[:, None, 128:256].broadcast_to([128, 2, 128]))
                pend.append((t, p, at, c0e))
                if len(pend) > CFG["acc_lag"]:
                    emit_acc(J, accs, pend.pop(0), n_t)
                unit_i += 1
                # proportional filler spreading (padded so some remain for the
                # pair-flush stalls)
                while fill_i < n_fill and fill_i * (n_units + 5) < unit_i * n_fill:
                    fillers[fill_i]()
                    fill_i += 1
            # flush the lag, interleaving remaining fillers to cover Exp latency
            while pend:
                emit_acc(J, accs, pend.pop(0), n_t)
                if pend and fill_i < n_fill:
                    fillers[fill_i]()
                    fill_i += 1
            normalize(J, accs, p)
        while fill_i < n_fill:
            fillers[fill_i]()
            fill_i += 1

    # ---- emission ----
    for _rep in range(repeat):
        # startup: kt-partial v(0..3) + qk(0), 8 concurrent psum banks
        v_ps = ([psum.tile([128, 512], F32, name="ps_acc", tag="acc", bufs=2)
                 for _ in range(2)]
                + [psum.tile([128, 512], F32, name="ps_op", tag="op", bufs=2)
                   for _ in range(2)])
        qk_ps = [psum.tile([128, 1024], F32, name="ps_sc", tag="sc",
                           bufs=CFG["sc_bufs"]) for _ in range(2)]
        for kt in range(KT):
            for i in range(4):
                v_mm(v_ps[i], i, kt)
            for ct in range(4):
                qk_ct_mm(qk_ps[ct // 2][:, (ct % 2) * 512:(ct % 2 + 1) * 512],
                         0, ct, [kt])
        for ct in range(4):
            qk_ct_drain(qk_ps[ct // 2][:, (ct % 2) * 512:(ct % 2 + 1) * 512],
                        0, ct, eng="scalar")
        for i in range(4):
            v_drain(v_ps[i], i)

        attention(0, fill_v(4) + fill_v(5) + fill_v(6) + fill_v(7)
                  + fill_qk_ct(1, 0) + fill_qk_ct(1, 1)
                  + fill_qk_ct(1, 2) + fill_qk_ct(1, 3))
        attention(1, fill_v(8) + fill_v(9) + fill_v(10) + fill_v(11)
                  + fill_qk_ct(2, 0) + fill_qk_ct(2, 1)
                  + fill_qk_ct(2, 2) + fill_qk_ct(2, 3))
        attention(2, fill_v(12) + fill_v(13) + fill_v(14) + fill_v(15)
                  + fill_qk_ct(3, 0) + fill_qk_ct(3, 1)
                  + fill_qk_ct(3, 2) + fill_qk_ct(3, 3)
                  + fill_op(0, 0) + fill_op(0, 1) + fill_op(0, 2) + fill_op(0, 3))
        held = []

        def hold_op3(dh):
            # kt2=0 half of op(3, nqs=0): runs during attention(3) p1 (saT pair
            # 0 rows are final); finished after the p1 normalize
            def f():
                op = psum.tile([128, 512], F32, name="ps_op", tag="op", bufs=2)
                nc.tensor.matmul(
                    op[:], saT_sb[:, 0, 3 * QB:3 * QB + 128],
                    wo_sb[:, 0, dh * 512:(dh + 1) * 512],
                    start=True, stop=False,
                    skip_group_check=True,
                )
                held.append((op, dh))
            return f

        attention(3, fill_op(1, 0) + fill_op(1, 1) + fill_op(1, 2) + fill_op(1, 3)
                  + fill_op(2, 0) + fill_op(2, 1) + fill_op(2, 2) + fill_op(2, 3)
                  + [hold_op3(0), hold_op3(1)])
        oe0 = sbp.tile([128, 1024], BF16, name="t_oe", tag="oe", bufs=CFG["oe_bufs"])
        for op, dh in held:
            nc.tensor.matmul(
                op[:], saT_sb[:, 1, 3 * QB:3 * QB + 128],
                wo_sb[:, 1, dh * 512:(dh + 1) * 512],
                start=False, stop=True,
                skip_group_check=True,
            )
            eng = nc.scalar.copy if dh == 0 else nc.vector.tensor_copy
            eng(oe0[:, dh * 512:(dh + 1) * 512], op[:])
            nc.sync.dma_start(out[3 * QB:3 * QB + 128, dh * 512:(dh + 1) * 512],
                              oe0[:, dh * 512:(dh + 1) * 512])
        for nqs in range(1, 4):
            for i, f in enumerate(fill_op(3, nqs,
                                          copy_eng=("scalar", "vector")[nqs % 2],
                                          dma_per_dh=True)):
                f()
        if dbg is not None:
            for kt2 in range(2):
                nc.sync.dma_start(dbg["saT"][kt2 * 128:(kt2 + 1) * 128, :],
                                  saT_sb[:, kt2, :].bitcast(F32))
            for ct in range(4):
                nc.sync.dma_start(dbg["qkT"][ct * 128:(ct + 1) * 128, :],
                                  qkT_sb[:, ct, :].bitcast(F32))
            nc.sync.dma_start(
                dbg["v65"][:, :],
                v65_sb[:, :, :].rearrange("p a b -> p (a b)").bitcast(F32))


def build(repeat=1, debug=False):
    nc = bacc.Bacc("TRN2", target_bir_lowering=False, debug=False,
                   num_devices=N_CORES)
    xT = nc.dram_tensor("xT", [D, N], BF16, kind="ExternalInput").ap()
    wqk = nc.dram_tensor("wqk", [D, 512], BF16, kind="ExternalInput").ap()
    wv = nc.dram_tensor("wv", [D, 256], BF16, kind="ExternalInput").ap()
    bqk = nc.dram_tensor("bqk", [4, 128], F32, kind="ExternalInput").ap()
    bv = nc.dram_tensor("bv", [1, 256], F32, kind="ExternalInput").ap()
    wo = nc.dram_tensor("wo", [256, 1024], F32R, kind="ExternalInput").ap()
    tri2 = nc.dram_tensor("tri2", [128, 256], F32R, kind="ExternalInput").ap()
    out = nc.dram_tensor("out", [N, D], BF16, kind="ExternalOutput").ap()
    dbg = None
    if debug:
        dbg = {
            "saT": nc.dram_tensor("dbg_saT", [256, N], F32, kind="ExternalOutput").ap(),
            "qkT": nc.dram_tensor("dbg_qkT", [512, N], F32, kind="ExternalOutput").ap(),
            "v65": nc.dram_tensor("dbg_v65", [128, NT * LH * 65], F32, kind="ExternalOutput").ap(),
        }

    with tile.TileContext(nc) as tc:
        with ExitStack() as ctx:
            _emit(nc, tc, ctx, (xT, wqk, wv, bqk, bv, wo, tri2, out), repeat=repeat,
                  dbg=dbg)
    nc.compile()
    return nc


def make_in_maps(x, Wqkv, bqkv, Wo):
    """Host-side sharding: per-core input dicts."""
    x = np.asarray(x, dtype=np.float32)
    Wqkv = np.asarray(Wqkv, dtype=np.float32)
    bqkv = np.asarray(bqkv, dtype=np.float32)
    Wo = np.asarray(Wo, dtype=np.float32)
    tri2 = np.concatenate(
        [np.zeros((128, 128), dtype=np.float32),
         np.triu(np.ones((128, 128), dtype=np.float32))], axis=1)
    in_maps = []
    for c in range(N_CORES):
        b, g = divmod(c, 4)
        hs = [4 * g + i for i in range(LH)]
        # source chunk order in Wqkv[h] columns: k (0:64), q (64:128), v (128:192)
        wqk_cols = []
        bqk_rows = []
        for p in range(2):
            hA, hB = hs[2 * p], hs[2 * p + 1]
            wqk_cols += [Wqkv[hA][:, 0:64], Wqkv[hB][:, 0:64]]    # k pair tile
            bqk_rows.append(np.concatenate([bqkv[hA][0:64], bqkv[hB][0:64]]))
            wqk_cols += [Wqkv[hA][:, 64:128], Wqkv[hB][:, 64:128]]  # q pair tile
            bqk_rows.append(np.concatenate([bqkv[hA][64:128], bqkv[hB][64:128]]))
        in_maps.append({
            "xT": np.ascontiguousarray(x[b].T).astype(ml_dtypes.bfloat16),
            "wqk": np.ascontiguousarray(
                np.concatenate(wqk_cols, axis=1)).astype(ml_dtypes.bfloat16),
            "wv": np.ascontiguousarray(
                np.concatenate([Wqkv[h][:, 128:192] for h in hs],
                               axis=1)).astype(ml_dtypes.bfloat16),
            "bqk": np.ascontiguousarray(np.stack(bqk_rows)),
            "bv": np.ascontiguousarray(
                np.concatenate([bqkv[h][128:192] for h in hs])[None, :]),
            "wo": np.ascontiguousarray(
                np.concatenate([Wo[h * HD:(h + 1) * HD, :] for h in hs], axis=0)),
            "tri2": tri2,
        })
    return in_maps


def kernel(x, Wqkv, bqkv, Wo, bo):
    if "nc" not in _CACHE:
        _CACHE["nc"] = build()
    nc = _CACHE["nc"]
    in_maps = make_in_maps(x, Wqkv, bqkv, Wo)
    res = bass_utils.run_bass_kernel_spmd(
        nc, in_maps, core_ids=list(range(N_CORES)))
    bo = np.asarray(bo, dtype=np.float32)
    full = np.empty((B, N, D), dtype=np.float32)
    for b in range(B):
        acc = res.results[4 * b]["out"].astype(np.float32).copy()
        for g in range(1, 4):
            acc += res.results[4 * b + g]["out"]
        full[b] = acc + bo[None, :]
    return full


# revision 12
# speedup vs baseline: 1.0326x; 1.0326x over previous
"""Causal self-attention (B=2, N=2048, D=1024, H=16, hd=64) on 8 trn2 NeuronCores.

Sharding: core c handles batch b = c//4 and 4 heads hs = [4*(c%4) .. 4*(c%4)+3]
(tensor-parallel over heads x data-parallel over batch). Each core computes its
heads' attention and a row-parallel partial of the output projection
(partial[n, :] = sum_{local heads} sa_h[n, :] @ Wo[h*64:(h+1)*64, :]); the host
sums the 4 partials per batch and adds the output bias.

Schedule (v2): kt-interleaved input DMA so the first projection matmuls start
~2us in (kt-partial accumulation of v/qk for query-block 0 as chunks land);
attention emitted as (t, pair) units with a 2-unit acc lag so the Exp latency
hides behind the next unit's score matmuls; later-block projections and the
output projection are woven between attention units so the PE never drains
while the Act engine (Exp) catches up. v-bias folded into the PSUM->SBUF drain
(DVE add with a partition-broadcast bias tile) instead of PE matmuls; the d=3
diagonal score/accumulate matmuls run 256 wide (fp32r <256-wide pays 4x) with
a [zeros|tri] mask zeroing the extra columns.
"""

import numpy as np
import ml_dtypes
from contextlib import ExitStack

import concourse.bass as bass
import concourse.tile as tile
from concourse import bacc, mybir
from concourse import bass_utils

F32 = mybir.dt.float32
F32R = mybir.dt.float32r
BF16 = mybir.dt.bfloat16
EXP = mybir.ActivationFunctionType.Exp

B, N, D, H, HD = 2, 2048, 1024, 16, 64
N_CORES = 8
LH = 4            # local heads per core
KT = D // 128     # 8 contraction k-tiles
NT = N // 128     # 16 n-tiles
NB = N // 512     # 4 n-blocks / q-blocks
QB = 512

_CACHE: dict = {}

CFG = {
    "at_bufs": 4,
    "oe_bufs": 3,
    "sc_bufs": 2,
    "acc_lag": 2,
    "act_primer": True,
    "n_warmup": 8,
}


def _emit(nc, tc, ctx, io, repeat=1, dbg=None):
    xT, wqk, wv, bqk, bv, wo, tri2, out = io

    persist = ctx.enter_context(tc.tile_pool(name="persist", bufs=1))
    sbp = ctx.enter_context(tc.tile_pool(name="work", bufs=1))
    psum = ctx.enter_context(tc.tile_pool(name="psum", bufs=1, space="PSUM"))

    # ---- persistent SBUF tensors ----
    xT_sb = persist.tile([128, KT, N], BF16)
    wqk_sb = persist.tile([128, KT, 512], BF16)
    wv_sb = persist.tile([128, KT, 256], BF16)
    wo_sb = persist.tile([128, 2, 1024], F32R)
    bqk_sb = persist.tile([128, 4], F32)
    bv_sb = persist.tile([1, 256], F32)
    bvbc_sb = persist.tile([128, 256], F32)
    ones_sb = persist.tile([1, 128], F32)
    tri2_sb = persist.tile([128, 256], F32R)
    qkT_sb = persist.tile([128, 4, N], F32R)
    v65_sb = persist.tile([128, NT, LH * 65], F32R)
    saT_sb = persist.tile([128, 2, N], F32R)

    # ---- input DMAs (SP issue order == DMA service order; HWDGE costs
    # ~625ns per instruction, so one instruction per tensor block) ----
    nc.sync.dma_start(wv_sb[:, :, :], wv.rearrange("(kt p) c -> p kt c", p=128))
    nc.sync.dma_start(xT_sb[:, :, 0:QB],
                      xT[:, 0:QB].rearrange("(kt p) c -> p kt c", p=128))
    nc.sync.dma_start(wqk_sb[:, :, :], wqk.rearrange("(kt p) c -> p kt c", p=128))
    nc.sync.dma_start(bqk_sb[:], bqk.rearrange("t p -> p t"))
    nc.sync.dma_start(bv_sb[:], bv[:])
    nc.sync.dma_start(tri2_sb[:], tri2[:])
    for nb in range(1, NB):
        nc.sync.dma_start(
            xT_sb[:, :, nb * QB:(nb + 1) * QB],
            xT[:, nb * QB:(nb + 1) * QB].rearrange("(kt p) c -> p kt c", p=128),
        )
    for kt2 in range(2):
        nc.sync.dma_start(wo_sb[:, kt2, :], wo[kt2 * 128:(kt2 + 1) * 128, :])

    nc.vector.memset(ones_sb[:], 1.0)
    # the +1 denominator row of v65: free index h*65+64 per (nt, h)
    ones_col = v65_sb[:, :, :].rearrange("p n (h c) -> p n h c", c=65)[:, :, :, 64:65]
    nc.vector.memset(ones_col.bitcast(F32), 1.0)
    # broadcast v-bias across partitions once (Pool is idle)
    nc.gpsimd.partition_broadcast(bvbc_sb[:], bv_sb[:])
    if CFG["act_primer"]:
        primer = sbp.tile([1, 1], F32, name="t_primer", tag="primer", bufs=1)
        nc.scalar.activation(primer[:], ones_sb[0:1, 0:1], EXP)
    if CFG["n_warmup"]:
        # ramp the PE p-state while the first input chunks stream in: dummy
        # rank-1 matmuls on a zeroed row into a never-read psum bank
        zz = persist.tile([1, 512], F32R)
        nc.vector.memset(zz[:].bitcast(F32), 0.0)
        wps = psum.tile([128, 512], F32, name="ps_op", tag="op", bufs=2)
        for i in range(CFG["n_warmup"]):
            nc.tensor.matmul(
                wps[:], ones_sb[0:1, 0:128].bitcast(F32R), zz[:],
                start=(i == 0), stop=(i == CFG["n_warmup"] - 1),
                skip_group_check=True,
            )

    # ---- phase pieces ----
    def v_mm(ps, nt, kt):
        # one 256-wide v matmul; a psum bank holds exactly one group
        # (start=True re-zeroes the whole 2KB zero region, so banks can't be
        # shared between accumulation groups)
        nc.tensor.matmul(
            ps[:, 0:256],
            xT_sb[:, kt, nt * 128:(nt + 1) * 128],
            wv_sb[:, kt, :],
            start=(kt == 0), stop=(kt == KT - 1),
            skip_group_check=True,
        )

    def v_drain(ps, nt):
        # psum -> v65 with the v-bias folded in (bias varies along free dim)
        dst = v65_sb[:, nt, :].rearrange("p (h c) -> p h c", c=65)[:, :, 0:64]
        src = ps[:, 0:256].rearrange("p (h c) -> p h c", c=64)
        bia = bvbc_sb[:, :].rearrange("p (h c) -> p h c", c=64)
        nc.vector.tensor_add(dst, src, bia)

    def qk_ct_mm(ps, nb, ct, kts):
        for kt in kts:
            nc.tensor.matmul(
                ps[:], wqk_sb[:, kt, ct * 128:(ct + 1) * 128],
                xT_sb[:, kt, nb * QB:(nb + 1) * QB],
                start=(kt == 0), stop=(kt == KT - 1),
                skip_group_check=True,
            )

    def qk_ct_drain(ps, nb, ct, eng="vector"):
        if eng == "scalar":
            nc.scalar.activation(
                qkT_sb[:, ct, nb * QB:(nb + 1) * QB], ps[:],
                mybir.ActivationFunctionType.Identity,
                bias=bqk_sb[:, ct:ct + 1])
        else:
            nc.vector.tensor_scalar_add(
                qkT_sb[:, ct, nb * QB:(nb + 1) * QB], ps[:], bqk_sb[:, ct:ct + 1])

    # steady-state filler pieces (each ~0.4-1 us of PE work)
    def fill_v(nt):
        ps = psum.tile([128, 512], F32, name="ps_op", tag="op", bufs=2)
        def h1():
            for kt in range(4):
                v_mm(ps, nt, kt)
        def h2():
            for kt in range(4, KT):
                v_mm(ps, nt, kt)
            v_drain(ps, nt)
        return [h1, h2]

    def fill_qk_ct(nb, ct):
        ps = psum.tile([128, 512], F32, name="ps_op", tag="op", bufs=2)
        def h1():
            qk_ct_mm(ps, nb, ct, range(4))
        def h2():
            qk_ct_mm(ps, nb, ct, range(4, KT))
            qk_ct_drain(ps, nb, ct)
        return [h1, h2]

    def fill_op(J, nqs, copy_eng="vector", dma_per_dh=False):
        # one output-projection row-block piece: both dh halves -> one DMA
        r0 = J * QB + nqs * 128
        oe = sbp.tile([128, 1024], BF16, name="t_oe", tag="oe", bufs=CFG["oe_bufs"])
        def piece(dh):
            def f():
                op = psum.tile([128, 512], F32, name="ps_op", tag="op", bufs=2)
                for kt2 in range(2):
                    nc.tensor.matmul(
                        op[:], saT_sb[:, kt2, r0:r0 + 128],
                        wo_sb[:, kt2, dh * 512:(dh + 1) * 512],
                        start=(kt2 == 0), stop=(kt2 == 1),
                        skip_group_check=True,
                    )
                if copy_eng == "scalar":
                    nc.scalar.copy(oe[:, dh * 512:(dh + 1) * 512], op[:])
                else:
                    nc.vector.tensor_copy(oe[:, dh * 512:(dh + 1) * 512], op[:])
                if dma_per_dh:
                    nc.sync.dma_start(out[r0:r0 + 128, dh * 512:(dh + 1) * 512],
                                      oe[:, dh * 512:(dh + 1) * 512])
                elif dh == 1:
                    nc.sync.dma_start(out[r0:r0 + 128, :], oe[:])
            return f
        return [piece(0), piece(1)]

    # ---- attention ----
    def emit_acc(J, accs, unit, n_t):
        t, p, at, c0e = unit
        for s in range(2):
            nc.tensor.matmul(
                accs[p][s][0:65, c0e:512],
                v65_sb[:, t, (2 * p + s) * 65:(2 * p + s) * 65 + 65],
                at[:, s * 512 + c0e:(s + 1) * 512],
                start=(t == 0), stop=(t == n_t - 1),
                skip_group_check=True,
            )

    def normalize(J, accs, p):
        for s in range(2):
            rc = sbp.tile([1, 512], F32, name="t_rc", tag="rc", bufs=2)
            nc.vector.reciprocal(rc[:], accs[p][s][64:65, :])
            bc = sbp.tile([64, 512], F32, name="t_bc", tag="bc", bufs=2)
            nc.gpsimd.partition_broadcast(bc[:], rc[:])
            nc.vector.tensor_mul(
                saT_sb[s * 64:(s + 1) * 64, p, J * QB:(J + 1) * QB],
                accs[p][s][0:64, :], bc[:],
            )

    def attention(J, fillers):
        n_t = 4 * J + 4
        n_units = n_t * 2
        n_fill = len(fillers)
        fill_i = 0
        unit_i = 0
        for p in range(2):
            accs = {p: [psum.tile([128, 512], F32, name="ps_acc", tag="acc",
                                  bufs=2) for _ in range(2)]}
            pend = []
            for t in range(n_t):
                d = t - 4 * J
                c0 = max(d, 0) * 128
                c0e = min(c0, 256)
                sc = psum.tile([128, 1024], F32, name="ps_sc", tag="sc",
                               bufs=CFG["sc_bufs"])
                for s in range(2):
                    nc.tensor.matmul(
                        sc[:, s * 512 + c0e:(s + 1) * 512],
                        qkT_sb[s * 64:(s + 1) * 64, 2 * p, t * 128:(t + 1) * 128],
                        qkT_sb[s * 64:(s + 1) * 64, 2 * p + 1, J * QB + c0e:(J + 1) * QB],
                        start=True, stop=True,
                    )
                at = sbp.tile([128, 1024], F32R, name="t_at", tag="at",
                              bufs=CFG["at_bufs"])
                nc.scalar.activation(at[:, c0e:1024], sc[:, c0e:1024],
                                     EXP, scale=0.125)
                if d == 3:
                    atv = at[:, 256:1024].rearrange(
                        "p (s c) -> p s c", c=256)[:, ::2, :]
                    nc.vector.tensor_mul(
                        atv, atv, tri2_sb[:, None, :].broadcast_to([128, 2, 256]))
                elif d >= 0:
                    atv = at[:, c0:c0 + 640].rearrange(
                        "p (s c) -> p s c", c=128)[:, ::4, :]
                    nc.vector.tensor_mul(
                        atv, atv,
                        tri2_sb[:, None, 128:256].broadcast_to([128, 2, 128]))
                pend.append((t, p, at, c0e))
                if len(pend) > CFG["acc_lag"]:
                    emit_acc(J, accs, pend.pop(0), n_t)
                unit_i += 1
                # proportional filler spreading (padded so some remain for the
                # pair-flush stalls)
                while fill_i < n_fill and fill_i * (n_units + 8) < unit_i * n_fill:
                    fillers[fill_i]()
                    fill_i += 1
            # flush the lag, interleaving remaining fillers to cover Exp latency
            while pend:
                emit_acc(J, accs, pend.pop(0), n_t)
                if pend and fill_i < n_fill:
                    fillers[fill_i]()
                    fill_i += 1
            normalize(J, accs, p)
        while fill_i < n_fill:
            fillers[fill_i]()
            fill_i += 1

    # ---- emission ----
    for _rep in range(repeat):
        # startup: kt-partial v(0..3) + qk(0), 8 concurrent psum banks
        v_ps = ([psum.tile([128, 512], F32, name="ps_acc", tag="acc", bufs=2)
                 for _ in range(2)]
                + [psum.tile([128, 512], F32, name="ps_op", tag="op", bufs=2)
                   for _ in range(2)])
        qk_ps = [psum.tile([128, 1024], F32, name="ps_sc", tag="sc",
                           bufs=CFG["sc_bufs"]) for _ in range(2)]
        for kt in range(KT):
            for i in range(4):
                v_mm(v_ps[i], i, kt)
            for ct in range(4):
                qk_ct_mm(qk_ps[ct // 2][:, (ct % 2) * 512:(ct % 2 + 1) * 512],
                         0, ct, [kt])
        for ct in range(4):
            qk_ct_drain(qk_ps[ct // 2][:, (ct % 2) * 512:(ct % 2 + 1) * 512],
                        0, ct, eng="scalar")
        for i in range(4):
            v_drain(v_ps[i], i)

        attention(0, fill_v(4) + fill_v(5) + fill_v(6) + fill_v(7)
                  + fill_qk_ct(1, 0) + fill_qk_ct(1, 1)
                  + fill_qk_ct(1, 2) + fill_qk_ct(1, 3))
        attention(1, fill_v(8) + fill_v(9) + fill_v(10) + fill_v(11)
                  + fill_qk_ct(2, 0) + fill_qk_ct(2, 1)
                  + fill_qk_ct(2, 2) + fill_qk_ct(2, 3))
        attention(2, fill_v(12) + fill_v(13) + fill_v(14) + fill_v(15)
                  + fill_qk_ct(3, 0) + fill_qk_ct(3, 1)
                  + fill_qk_ct(3, 2) + fill_qk_ct(3, 3)
                  + fill_op(0, 0) + fill_op(0, 1) + fill_op(0, 2) + fill_op(0, 3))
        held = []

        def hold_op3(dh):
            # kt2=0 half of op(3, nqs=0): runs during attention(3) p1 (saT pair
            # 0 rows are final); finished after the p1 normalize
            def f():
                op = psum.tile([128, 512], F32, name="ps_op", tag="op", bufs=2)
                nc.tensor.matmul(
                    op[:], saT_sb[:, 0, 3 * QB:3 * QB + 128],
                    wo_sb[:, 0, dh * 512:(dh + 1) * 512],
                    start=True, stop=False,
                    skip_group_check=True,
                )
                held.append((op, dh))
            return f

        attention(3, fill_op(1, 0) + fill_op(1, 1) + fill_op(1, 2) + fill_op(1, 3)
                  + fill_op(2, 0) + fill_op(2, 1) + fill_op(2, 2) + fill_op(2, 3)
                  + [hold_op3(0), hold_op3(1)])
        oe0 = sbp.tile([128, 1024], BF16, name="t_oe", tag="oe", bufs=CFG["oe_bufs"])
        for op, dh in held:
            nc.tensor.matmul(
                op[:], saT_sb[:, 1, 3 * QB:3 * QB + 128],
                wo_sb[:, 1, dh * 512:(dh + 1) * 512],
                start=False, stop=True,
                skip_group_check=True,
            )
            eng = nc.scalar.copy if dh == 0 else nc.vector.tensor_copy
            eng(oe0[:, dh * 512:(dh + 1) * 512], op[:])
            nc.sync.dma_start(out[3 * QB:3 * QB + 128, dh * 512:(dh + 1) * 512],
                              oe0[:, dh * 512:(dh + 1) * 512])
        for nqs in range(1, 4):
            for i, f in enumerate(fill_op(3, nqs,
                                          copy_eng=("scalar", "vector")[nqs % 2],
                                          dma_per_dh=True)):
                f()
        if dbg is not None:
            for kt2 in range(2):
                nc.sync.dma_start(dbg["saT"][kt2 * 128:(kt2 + 1) * 128, :],
                                  saT_sb[:, kt2, :].bitcast(F32))
            for ct in range(4):
                nc.sync.dma_start(dbg["qkT"][ct * 128:(ct + 1) * 128, :],
                                  qkT_sb[:, ct, :].bitcast(F32))
            nc.sync.dma_start(
                dbg["v65"][:, :],
                v65_sb[:, :, :].rearrange("p a b -> p (a b)").bitcast(F32))


def build(repeat=1, debug=False):
    nc = bacc.Bacc("TRN2", target_bir_lowering=False, debug=False,
                   num_devices=N_CORES)
    xT = nc.dram_tensor("xT", [D, N], BF16, kind="ExternalInput").ap()
    wqk = nc.dram_tensor("wqk", [D, 512], BF16, kind="ExternalInput").ap()
    wv = nc.dram_tensor("wv", [D, 256], BF16, kind="ExternalInput").ap()
    bqk = nc.dram_tensor("bqk", [4, 128], F32, kind="ExternalInput").ap()
    bv = nc.dram_tensor("bv", [1, 256], F32, kind="ExternalInput").ap()
    wo = nc.dram_tensor("wo", [256, 1024], F32R, kind="ExternalInput").ap()
    tri2 = nc.dram_tensor("tri2", [128, 256], F32R, kind="ExternalInput").ap()
    out = nc.dram_tensor("out", [N, D], BF16, kind="ExternalOutput").ap()
    dbg = None
    if debug:
        dbg = {
            "saT": nc.dram_tensor("dbg_saT", [256, N], F32, kind="ExternalOutput").ap(),
            "qkT": nc.dram_tensor("dbg_qkT", [512, N], F32, kind="ExternalOutput").ap(),
            "v65": nc.dram_tensor("dbg_v65", [128, NT * LH * 65], F32, kind="ExternalOutput").ap(),
        }

    with tile.TileContext(nc) as tc:
        with ExitStack() as ctx:
            _emit(nc, tc, ctx, (xT, wqk, wv, bqk, bv, wo, tri2, out), repeat=repeat,
                  dbg=dbg)
    nc.compile()
    return nc


def make_in_maps(x, Wqkv, bqkv, Wo):
    """Host-side sharding: per-core input dicts."""
    x = np.asarray(x, dtype=np.float32)
    Wqkv = np.asarray(Wqkv, dtype=np.float32)
    bqkv = np.asarray(bqkv, dtype=np.float32)
    Wo = np.asarray(Wo, dtype=np.float32)
    tri2 = np.concatenate(
        [np.zeros((128, 128), dtype=np.float32),
         np.triu(np.ones((128, 128), dtype=np.float32))], axis=1)
    in_maps = []
    for c in range(N_CORES):
        b, g = divmod(c, 4)
        hs = [4 * g + i for i in range(LH)]
        # source chunk order in Wqkv[h] columns: k (0:64), q (64:128), v (128:192)
        wqk_cols = []
        bqk_rows = []
        for p in range(2):
            hA, hB = hs[2 * p], hs[2 * p + 1]
            wqk_cols += [Wqkv[hA][:, 0:64], Wqkv[hB][:, 0:64]]    # k pair tile
            bqk_rows.append(np.concatenate([bqkv[hA][0:64], bqkv[hB][0:64]]))
            wqk_cols += [Wqkv[hA][:, 64:128], Wqkv[hB][:, 64:128]]  # q pair tile
            bqk_rows.append(np.concatenate([bqkv[hA][64:128], bqkv[hB][64:128]]))
        in_maps.append({
            "xT": np.ascontiguousarray(x[b].T).astype(ml_dtypes.bfloat16),
            "wqk": np.ascontiguousarray(
                np.concatenate(wqk_cols, axis=1)).astype(ml_dtypes.bfloat16),
            "wv": np.ascontiguousarray(
                np.concatenate([Wqkv[h][:, 128:192] for h in hs],
                               axis=1)).astype(ml_dtypes.bfloat16),
            "bqk": np.ascontiguousarray(np.stack(bqk_rows)),
            "bv": np.ascontiguousarray(
                np.concatenate([bqkv[h][128:192] for h in hs])[None, :]),
            "wo": np.ascontiguousarray(
                np.concatenate([Wo[h * HD:(h + 1) * HD, :] for h in hs], axis=0)),
            "tri2": tri2,
        })
    return in_maps


def kernel(x, Wqkv, bqkv, Wo, bo):
    if "nc" not in _CACHE:
        _CACHE["nc"] = build()
    nc = _CACHE["nc"]
    in_maps = make_in_maps(x, Wqkv, bqkv, Wo)
    res = bass_utils.run_bass_kernel_spmd(
        nc, in_maps, core_ids=list(range(N_CORES)))
    bo = np.asarray(bo, dtype=np.float32)
    full = np.empty((B, N, D), dtype=np.float32)
    for b in range(B):
        acc = res.results[4 * b]["out"].astype(np.float32).copy()
        for g in range(1, 4):
            acc += res.results[4 * b + g]["out"]
        full[b] = acc + bo[None, :]
    return full


# revision 13
# speedup vs baseline: 1.0526x; 1.0194x over previous
"""Causal self-attention (B=2, N=2048, D=1024, H=16, hd=64) on 8 trn2 NeuronCores.

Sharding: core c handles batch b = c//4 and 4 heads hs = [4*(c%4) .. 4*(c%4)+3]
(tensor-parallel over heads x data-parallel over batch). Each core computes its
heads' attention and a row-parallel partial of the output projection
(partial[n, :] = sum_{local heads} sa_h[n, :] @ Wo[h*64:(h+1)*64, :]); the host
sums the 4 partials per batch and adds the output bias.

Schedule (v2): kt-interleaved input DMA so the first projection matmuls start
~2us in (kt-partial accumulation of v/qk for query-block 0 as chunks land);
attention emitted as (t, pair) units with a 2-unit acc lag so the Exp latency
hides behind the next unit's score matmuls; later-block projections and the
output projection are woven between attention units so the PE never drains
while the Act engine (Exp) catches up. v-bias folded into the PSUM->SBUF drain
(DVE add with a partition-broadcast bias tile) instead of PE matmuls; the d=3
diagonal score/accumulate matmuls run 256 wide (fp32r <256-wide pays 4x) with
a [zeros|tri] mask zeroing the extra columns.
"""

import numpy as np
import ml_dtypes
from contextlib import ExitStack

import concourse.bass as bass
import concourse.tile as tile
from concourse import bacc, mybir
from concourse import bass_utils

F32 = mybir.dt.float32
F32R = mybir.dt.float32r
BF16 = mybir.dt.bfloat16
EXP = mybir.ActivationFunctionType.Exp

B, N, D, H, HD = 2, 2048, 1024, 16, 64
N_CORES = 8
LH = 4            # local heads per core
KT = D // 128     # 8 contraction k-tiles
NT = N // 128     # 16 n-tiles
NB = N // 512     # 4 n-blocks / q-blocks
QB = 512

_CACHE: dict = {}

CFG = {
    "at_bufs": 4,
    "oe_bufs": 4,
    "sc_bufs": 2,
    "acc_lag": 2,
    "act_primer": True,
    "n_warmup": 14,
}


def _emit(nc, tc, ctx, io, repeat=1, dbg=None):
    xT, wqk, wv, bqk, bv, wo, tri2, out = io

    persist = ctx.enter_context(tc.tile_pool(name="persist", bufs=1))
    sbp = ctx.enter_context(tc.tile_pool(name="work", bufs=1))
    psum = ctx.enter_context(tc.tile_pool(name="psum", bufs=1, space="PSUM"))

    # ---- persistent SBUF tensors ----
    xT_sb = persist.tile([128, KT, N], BF16)
    wqk_sb = persist.tile([128, KT, 512], BF16)
    wv_sb = persist.tile([128, KT, 256], BF16)
    wo_sb = persist.tile([128, 2, 1024], F32R)
    bqk_sb = persist.tile([128, 4], F32)
    bv_sb = persist.tile([1, 256], F32)
    bvbc_sb = persist.tile([128, 256], F32)
    ones_sb = persist.tile([1, 128], F32)
    tri2_sb = persist.tile([128, 256], F32R)
    qkT_sb = persist.tile([128, 4, N], F32R)
    v65_sb = persist.tile([128, NT, LH * 65], F32R)
    saT_sb = persist.tile([128, 2, N], F32R)

    # ---- input DMAs (SP issue order == DMA service order; HWDGE costs
    # ~625ns per instruction, so one instruction per tensor block) ----
    nc.sync.dma_start(wv_sb[:, :, :], wv.rearrange("(kt p) c -> p kt c", p=128))
    nc.sync.dma_start(xT_sb[:, :, 0:QB],
                      xT[:, 0:QB].rearrange("(kt p) c -> p kt c", p=128))
    nc.sync.dma_start(wqk_sb[:, :, :], wqk.rearrange("(kt p) c -> p kt c", p=128))
    nc.sync.dma_start(bqk_sb[:], bqk.rearrange("t p -> p t"))
    nc.sync.dma_start(bv_sb[:], bv[:])
    nc.sync.dma_start(tri2_sb[:], tri2[:])
    for nb in range(1, NB):
        nc.sync.dma_start(
            xT_sb[:, :, nb * QB:(nb + 1) * QB],
            xT[:, nb * QB:(nb + 1) * QB].rearrange("(kt p) c -> p kt c", p=128),
        )
    for kt2 in range(2):
        nc.sync.dma_start(wo_sb[:, kt2, :], wo[kt2 * 128:(kt2 + 1) * 128, :])

    nc.vector.memset(ones_sb[:], 1.0)
    # the +1 denominator row of v65: free index h*65+64 per (nt, h)
    ones_col = v65_sb[:, :, :].rearrange("p n (h c) -> p n h c", c=65)[:, :, :, 64:65]
    nc.vector.memset(ones_col.bitcast(F32), 1.0)
    # broadcast v-bias across partitions once (Pool is idle)
    nc.gpsimd.partition_broadcast(bvbc_sb[:], bv_sb[:])
    if CFG["act_primer"]:
        primer = sbp.tile([1, 1], F32, name="t_primer", tag="primer", bufs=1)
        nc.scalar.activation(primer[:], ones_sb[0:1, 0:1], EXP)
    if CFG["n_warmup"]:
        # ramp the PE p-state while the first input chunks stream in: dummy
        # rank-1 matmuls on a zeroed row into a never-read psum bank
        zz = persist.tile([1, 512], F32R)
        nc.vector.memset(zz[:].bitcast(F32), 0.0)
        wps = psum.tile([128, 512], F32, name="ps_op", tag="op", bufs=2)
        for i in range(CFG["n_warmup"]):
            nc.tensor.matmul(
                wps[:], ones_sb[0:1, 0:128].bitcast(F32R), zz[:],
                start=(i == 0), stop=(i == CFG["n_warmup"] - 1),
                skip_group_check=True,
            )

    # ---- phase pieces ----
    def v_mm(ps, nt, kt):
        # one 256-wide v matmul; a psum bank holds exactly one group
        # (start=True re-zeroes the whole 2KB zero region, so banks can't be
        # shared between accumulation groups)
        nc.tensor.matmul(
            ps[:, 0:256],
            xT_sb[:, kt, nt * 128:(nt + 1) * 128],
            wv_sb[:, kt, :],
            start=(kt == 0), stop=(kt == KT - 1),
            skip_group_check=True,
        )

    def v_drain(ps, nt):
        # psum -> v65 with the v-bias folded in (bias varies along free dim)
        dst = v65_sb[:, nt, :].rearrange("p (h c) -> p h c", c=65)[:, :, 0:64]
        src = ps[:, 0:256].rearrange("p (h c) -> p h c", c=64)
        bia = bvbc_sb[:, :].rearrange("p (h c) -> p h c", c=64)
        nc.vector.tensor_add(dst, src, bia)

    def qk_ct_mm(ps, nb, ct, kts):
        for kt in kts:
            nc.tensor.matmul(
                ps[:], wqk_sb[:, kt, ct * 128:(ct + 1) * 128],
                xT_sb[:, kt, nb * QB:(nb + 1) * QB],
                start=(kt == 0), stop=(kt == KT - 1),
                skip_group_check=True,
            )

    def qk_ct_drain(ps, nb, ct, eng="vector"):
        if eng == "scalar":
            nc.scalar.activation(
                qkT_sb[:, ct, nb * QB:(nb + 1) * QB], ps[:],
                mybir.ActivationFunctionType.Identity,
                bias=bqk_sb[:, ct:ct + 1])
        else:
            nc.vector.tensor_scalar_add(
                qkT_sb[:, ct, nb * QB:(nb + 1) * QB], ps[:], bqk_sb[:, ct:ct + 1])

    # steady-state filler pieces (each ~0.4-1 us of PE work)
    def fill_v(nt):
        ps = psum.tile([128, 512], F32, name="ps_op", tag="op", bufs=2)
        def h1():
            for kt in range(4):
                v_mm(ps, nt, kt)
        def h2():
            for kt in range(4, KT):
                v_mm(ps, nt, kt)
            v_drain(ps, nt)
        return [h1, h2]

    def fill_qk_ct(nb, ct):
        ps = psum.tile([128, 512], F32, name="ps_op", tag="op", bufs=2)
        def h1():
            qk_ct_mm(ps, nb, ct, range(4))
        def h2():
            qk_ct_mm(ps, nb, ct, range(4, KT))
            # Act has slack during att(0)/att(1); DVE is the tighter engine
            qk_ct_drain(ps, nb, ct, eng="scalar" if nb <= 2 else "vector")
        return [h1, h2]

    def fill_op(J, nqs, copy_eng="vector", dma_per_dh=False):
        # one output-projection row-block piece: both dh halves -> one DMA
        r0 = J * QB + nqs * 128
        oe = sbp.tile([128, 1024], BF16, name="t_oe", tag="oe", bufs=CFG["oe_bufs"])
        def piece(dh):
            def f():
                op = psum.tile([128, 512], F32, name="ps_op", tag="op", bufs=2)
                for kt2 in range(2):
                    nc.tensor.matmul(
                        op[:], saT_sb[:, kt2, r0:r0 + 128],
                        wo_sb[:, kt2, dh * 512:(dh + 1) * 512],
                        start=(kt2 == 0), stop=(kt2 == 1),
                        skip_group_check=True,
                    )
                if copy_eng == "scalar":
                    nc.scalar.copy(oe[:, dh * 512:(dh + 1) * 512], op[:])
                else:
                    nc.vector.tensor_copy(oe[:, dh * 512:(dh + 1) * 512], op[:])
                if dma_per_dh:
                    nc.sync.dma_start(out[r0:r0 + 128, dh * 512:(dh + 1) * 512],
                                      oe[:, dh * 512:(dh + 1) * 512])
                elif dh == 1:
                    nc.sync.dma_start(out[r0:r0 + 128, :], oe[:])
            return f
        return [piece(0), piece(1)]

    # ---- attention ----
    def emit_acc(J, accs, unit, n_t):
        t, p, at, c0e = unit
        for s in range(2):
            nc.tensor.matmul(
                accs[p][s][0:65, c0e:512],
                v65_sb[:, t, (2 * p + s) * 65:(2 * p + s) * 65 + 65],
                at[:, s * 512 + c0e:(s + 1) * 512],
                start=(t == 0), stop=(t == n_t - 1),
                skip_group_check=True,
            )

    def normalize(J, accs, p, split=False):
        rcs = []
        for s in range(2):
            rc = sbp.tile([1, 512], F32, name="t_rc", tag="rc", bufs=2)
            nc.vector.reciprocal(rc[:], accs[p][s][64:65, :])
            bc = sbp.tile([64, 512], F32, name="t_bc", tag="bc", bufs=2)
            nc.gpsimd.partition_broadcast(bc[:], rc[:])
            rcs.append(bc)
        halves = ((0, 256), (256, 512)) if split else ((0, 512),)
        for a, b in halves:
            for s in range(2):
                nc.vector.tensor_mul(
                    saT_sb[s * 64:(s + 1) * 64, p, J * QB + a:J * QB + b],
                    accs[p][s][0:64, a:b], rcs[s][:, a:b],
                )

    def attention(J, fillers):
        n_t = 4 * J + 4
        n_units = n_t * 2
        n_fill = len(fillers)
        fill_i = 0
        unit_i = 0
        for p in range(2):
            accs = {p: [psum.tile([128, 512], F32, name="ps_acc", tag="acc",
                                  bufs=2) for _ in range(2)]}
            pend = []
            for t in range(n_t):
                d = t - 4 * J
                c0 = max(d, 0) * 128
                c0e = min(c0, 256)
                sc = psum.tile([128, 1024], F32, name="ps_sc", tag="sc",
                               bufs=CFG["sc_bufs"])
                for s in range(2):
                    nc.tensor.matmul(
                        sc[:, s * 512 + c0e:(s + 1) * 512],
                        qkT_sb[s * 64:(s + 1) * 64, 2 * p, t * 128:(t + 1) * 128],
                        qkT_sb[s * 64:(s + 1) * 64, 2 * p + 1, J * QB + c0e:(J + 1) * QB],
                        start=True, stop=True,
                    )
                at = sbp.tile([128, 1024], F32R, name="t_at", tag="at",
                              bufs=CFG["at_bufs"])
                nc.scalar.activation(at[:, c0e:1024], sc[:, c0e:1024],
                                     EXP, scale=0.125)
                if d == 3:
                    atv = at[:, 256:1024].rearrange(
                        "p (s c) -> p s c", c=256)[:, ::2, :]
                    nc.vector.tensor_mul(
                        atv, atv, tri2_sb[:, None, :].broadcast_to([128, 2, 256]))
                elif d >= 0:
                    atv = at[:, c0:c0 + 640].rearrange(
                        "p (s c) -> p s c", c=128)[:, ::4, :]
                    nc.vector.tensor_mul(
                        atv, atv,
                        tri2_sb[:, None, 128:256].broadcast_to([128, 2, 128]))
                pend.append((t, p, at, c0e))
                if len(pend) > CFG["acc_lag"]:
                    emit_acc(J, accs, pend.pop(0), n_t)
                unit_i += 1
                # proportional filler spreading (padded so some remain for the
                # pair-flush stalls)
                while fill_i < n_fill and fill_i * (n_units + 8) < unit_i * n_fill:
                    fillers[fill_i]()
                    fill_i += 1
            # flush the lag, interleaving remaining fillers to cover Exp latency
            while pend:
                emit_acc(J, accs, pend.pop(0), n_t)
                if pend and fill_i < n_fill:
                    fillers[fill_i]()
                    fill_i += 1
            normalize(J, accs, p, split=(J == 3 and p == 1))
        while fill_i < n_fill:
            fillers[fill_i]()
            fill_i += 1

    # ---- emission ----
    for _rep in range(repeat):
        # startup: kt-partial v(0..3) + qk(0), 8 concurrent psum banks
        v_ps = ([psum.tile([128, 512], F32, name="ps_acc", tag="acc", bufs=2)
                 for _ in range(2)]
                + [psum.tile([128, 512], F32, name="ps_op", tag="op", bufs=2)
                   for _ in range(2)])
        qk_ps = [psum.tile([128, 1024], F32, name="ps_sc", tag="sc",
                           bufs=CFG["sc_bufs"]) for _ in range(2)]
        for kt in range(KT):
            for i in range(4):
                v_mm(v_ps[i], i, kt)
            for ct in range(4):
                qk_ct_mm(qk_ps[ct // 2][:, (ct % 2) * 512:(ct % 2 + 1) * 512],
                         0, ct, [kt])
        for ct in range(4):
            qk_ct_drain(qk_ps[ct // 2][:, (ct % 2) * 512:(ct % 2 + 1) * 512],
                        0, ct, eng="scalar")
        for i in range(4):
            v_drain(v_ps[i], i)

        attention(0, fill_v(4) + fill_v(5) + fill_v(6) + fill_v(7)
                  + fill_qk_ct(1, 0) + fill_qk_ct(1, 1)
                  + fill_qk_ct(1, 2) + fill_qk_ct(1, 3))
        attention(1, fill_v(8) + fill_v(9) + fill_v(10) + fill_v(11)
                  + fill_qk_ct(2, 0) + fill_qk_ct(2, 1)
                  + fill_qk_ct(2, 2) + fill_qk_ct(2, 3))
        attention(2, fill_v(12) + fill_v(13) + fill_v(14) + fill_v(15)
                  + fill_qk_ct(3, 0) + fill_qk_ct(3, 1)
                  + fill_qk_ct(3, 2) + fill_qk_ct(3, 3)
                  + fill_op(0, 0) + fill_op(0, 1) + fill_op(0, 2) + fill_op(0, 3))
        held = []

        def hold_op3(dh):
            # kt2=0 half of op(3, nqs=0): runs during attention(3) p1 (saT pair
            # 0 rows are final); finished after the p1 normalize
            def f():
                op = psum.tile([128, 512], F32, name="ps_op", tag="op", bufs=2)
                nc.tensor.matmul(
                    op[:], saT_sb[:, 0, 3 * QB:3 * QB + 128],
                    wo_sb[:, 0, dh * 512:(dh + 1) * 512],
                    start=True, stop=False,
                    skip_group_check=True,
                )
                held.append((op, dh))
            return f

        attention(3, fill_op(1, 0) + fill_op(1, 1) + fill_op(1, 2) + fill_op(1, 3)
                  + fill_op(2, 0) + fill_op(2, 1) + fill_op(2, 2) + fill_op(2, 3)
                  + [hold_op3(0), hold_op3(1)])
        oe0 = sbp.tile([128, 1024], BF16, name="t_oe", tag="oe", bufs=CFG["oe_bufs"])
        for op, dh in held:
            nc.tensor.matmul(
                op[:], saT_sb[:, 1, 3 * QB:3 * QB + 128],
                wo_sb[:, 1, dh * 512:(dh + 1) * 512],
                start=False, stop=True,
                skip_group_check=True,
            )
            eng = nc.scalar.copy if dh == 0 else nc.vector.tensor_copy
            eng(oe0[:, dh * 512:(dh + 1) * 512], op[:])
            nc.sync.dma_start(out[3 * QB:3 * QB + 128, dh * 512:(dh + 1) * 512],
                              oe0[:, dh * 512:(dh + 1) * 512])
        for nqs in range(1, 4):
            for i, f in enumerate(fill_op(3, nqs,
                                          copy_eng=("scalar", "vector")[nqs % 2],
                                          dma_per_dh=True)):
                f()
        if dbg is not None:
            for kt2 in range(2):
                nc.sync.dma_start(dbg["saT"][kt2 * 128:(kt2 + 1) * 128, :],
                                  saT_sb[:, kt2, :].bitcast(F32))
            for ct in range(4):
                nc.sync.dma_start(dbg["qkT"][ct * 128:(ct + 1) * 128, :],
                                  qkT_sb[:, ct, :].bitcast(F32))
            nc.sync.dma_start(
                dbg["v65"][:, :],
                v65_sb[:, :, :].rearrange("p a b -> p (a b)").bitcast(F32))


def build(repeat=1, debug=False):
    nc = bacc.Bacc("TRN2", target_bir_lowering=False, debug=False,
                   num_devices=N_CORES)
    xT = nc.dram_tensor("xT", [D, N], BF16, kind="ExternalInput").ap()
    wqk = nc.dram_tensor("wqk", [D, 512], BF16, kind="ExternalInput").ap()
    wv = nc.dram_tensor("wv", [D, 256], BF16, kind="ExternalInput").ap()
    bqk = nc.dram_tensor("bqk", [4, 128], F32, kind="ExternalInput").ap()
    bv = nc.dram_tensor("bv", [1, 256], F32, kind="ExternalInput").ap()
    wo = nc.dram_tensor("wo", [256, 1024], F32R, kind="ExternalInput").ap()
    tri2 = nc.dram_tensor("tri2", [128, 256], F32R, kind="ExternalInput").ap()
    out = nc.dram_tensor("out", [N, D], BF16, kind="ExternalOutput").ap()
    dbg = None
    if debug:
        dbg = {
            "saT": nc.dram_tensor("dbg_saT", [256, N], F32, kind="ExternalOutput").ap(),
            "qkT": nc.dram_tensor("dbg_qkT", [512, N], F32, kind="ExternalOutput").ap(),
            "v65": nc.dram_tensor("dbg_v65", [128, NT * LH * 65], F32, kind="ExternalOutput").ap(),
        }

    with tile.TileContext(nc) as tc:
        with ExitStack() as ctx:
            _emit(nc, tc, ctx, (xT, wqk, wv, bqk, bv, wo, tri2, out), repeat=repeat,
                  dbg=dbg)
    nc.compile()
    return nc


def make_in_maps(x, Wqkv, bqkv, Wo):
    """Host-side sharding: per-core input dicts."""
    x = np.asarray(x, dtype=np.float32)
    Wqkv = np.asarray(Wqkv, dtype=np.float32)
    bqkv = np.asarray(bqkv, dtype=np.float32)
    Wo = np.asarray(Wo, dtype=np.float32)
    tri2 = np.concatenate(
        [np.zeros((128, 128), dtype=np.float32),
         np.triu(np.ones((128, 128), dtype=np.float32))], axis=1)
    in_maps = []
    for c in range(N_CORES):
        b, g = divmod(c, 4)
        hs = [4 * g + i for i in range(LH)]
        # source chunk order in Wqkv[h] columns: k (0:64), q (64:128), v (128:192)
        wqk_cols = []
        bqk_rows = []
        for p in range(2):
            hA, hB = hs[2 * p], hs[2 * p + 1]
            wqk_cols += [Wqkv[hA][:, 0:64], Wqkv[hB][:, 0:64]]    # k pair tile
            bqk_rows.append(np.concatenate([bqkv[hA][0:64], bqkv[hB][0:64]]))
            wqk_cols += [Wqkv[hA][:, 64:128], Wqkv[hB][:, 64:128]]  # q pair tile
            bqk_rows.append(np.concatenate([bqkv[hA][64:128], bqkv[hB][64:128]]))
        in_maps.append({
            "xT": np.ascontiguousarray(x[b].T).astype(ml_dtypes.bfloat16),
            "wqk": np.ascontiguousarray(
                np.concatenate(wqk_cols, axis=1)).astype(ml_dtypes.bfloat16),
            "wv": np.ascontiguousarray(
                np.concatenate([Wqkv[h][:, 128:192] for h in hs],
                               axis=1)).astype(ml_dtypes.bfloat16),
            "bqk": np.ascontiguousarray(np.stack(bqk_rows)),
            "bv": np.ascontiguousarray(
                np.concatenate([bqkv[h][128:192] for h in hs])[None, :]),
            "wo": np.ascontiguousarray(
                np.concatenate([Wo[h * HD:(h + 1) * HD, :] for h in hs], axis=0)),
            "tri2": tri2,
        })
    return in_maps


def kernel(x, Wqkv, bqkv, Wo, bo):
    if "nc" not in _CACHE:
        _CACHE["nc"] = build()
    nc = _CACHE["nc"]
    in_maps = make_in_maps(x, Wqkv, bqkv, Wo)
    res = bass_utils.run_bass_kernel_spmd(
        nc, in_maps, core_ids=list(range(N_CORES)))
    bo = np.asarray(bo, dtype=np.float32)
    full = np.empty((B, N, D), dtype=np.float32)
    for b in range(B):
        acc = res.results[4 * b]["out"].astype(np.float32).copy()
        for g in range(1, 4):
            acc += res.results[4 * b + g]["out"]
        full[b] = acc + bo[None, :]
    return full
